# revision 9
# baseline (speedup 1.0000x reference)
"""Trainium2 Bass kernel for packed-sequence RNN (nn_RNN_60979945669189).

Strategy
--------
Sequences are sorted by length: global row j (sorted order) has length
L_j = 2*(256-j).  Batch is sharded 32 rows/core across 8 cores.  The
recurrence h_t = tanh(W h_{t-1} + U x_t + b) keeps h TRANSPOSED in SBUF
([h, b] layout, [128, 8*32] tile) so each step is 64 [128,128]x[128,32]
matmuls with W.T tiles as the bf16 stationary operand (FWL weight load).
U x_t + b ("g") has no sequential dependency and is precomputed on-device
into HBM with N=512 f32 matmuls.  The per-step sigmoid/classifier matmuls
are hoisted entirely out of the loop: o and y only survive at each row's
last active step, where they are pure functions of the final h.

SPMD uniformity: all cores run the identical 512-step program.  At every
odd step t exactly one global row finishes; every core copies its local
column c(t) = ((511-t)//2) % 32 of tanh(z_t) (f32) into epoch buffer
e(t) = (511-t)//64.  Core k's true finals are exactly epoch k (rows
32k..32k+31 finish at t = 2*(256-32k-c)-1, i.e. in epoch-k's t-window);
the host just selects epoch k from core k.  The tiny v/c epilogue is done
for all 8 epochs on every core and the host again selects epoch k.
"""

import sys

for _p in ("/opt/trn_rl_repo", "/opt/pypackages"):
    if _p not in sys.path:
        sys.path.insert(0, _p)

from contextlib import ExitStack

import ml_dtypes
import numpy as np

import concourse.bass as bass
import concourse.mybir as mybir
import concourse.tile as tile
from concourse import bacc

B = 256
T = 512
INPUT = 512
HIDDEN = 1024
HALF = HIDDEN // 2
OUT = 64
NCORES = 8
RPC = B // NCORES          # rows per core = 32
NSTEPS = T                 # uniform step count on every core
KH = HIDDEN // 128         # 8 h-chunks
KI = INPUT // 128          # 4 input chunks
KO = HALF // 128           # 4 o-chunks
F32 = mybir.dt.float32
BF16 = mybir.dt.bfloat16

GCHUNK = 16                # g-phase steps per chunk (N = GCHUNK*RPC = 512)


def _build_program(nsteps=NSTEPS):
    nc = bacc.Bacc("TRN2", target_bir_lowering=False, debug=False,
                   num_devices=NCORES)
    ngch = nsteps // GCHUNK

    xT = nc.dram_tensor("xT", [KI, 128, nsteps * RPC], F32,
                        kind="ExternalInput").ap()
    wT = nc.dram_tensor("wT", [HIDDEN, HIDDEN], BF16, kind="ExternalInput").ap()
    uT = nc.dram_tensor("uT", [INPUT, HIDDEN], F32, kind="ExternalInput").ap()
    vT = nc.dram_tensor("vT", [HIDDEN, HALF], F32, kind="ExternalInput").ap()
    cT = nc.dram_tensor("cT", [HALF, OUT], F32, kind="ExternalInput").ap()
    ubwb = nc.dram_tensor("ubwb", [128, KH], F32, kind="ExternalInput").ap()
    vb = nc.dram_tensor("vb", [128, KO], F32, kind="ExternalInput").ap()
    cb = nc.dram_tensor("cb", [OUT, 1], F32, kind="ExternalInput").ap()

    g_dram = nc.dram_tensor("g_scratch", [nsteps, 128, KH * RPC], F32).ap()

    snap_out = nc.dram_tensor("snap", [NCORES, 128, KH * RPC], F32,
                              kind="ExternalOutput").ap()
    y_out = nc.dram_tensor("y", [NCORES, OUT, RPC], F32,
                           kind="ExternalOutput").ap()

    with tile.TileContext(nc) as tc, ExitStack() as ctx:
        const = ctx.enter_context(tc.tile_pool(name="const", bufs=1))

        # resident weights / biases
        w_sb = []
        for k in range(KH):
            t_ = const.tile([128, HIDDEN], BF16, tag=f"w{k}")
            nc.sync.dma_start(t_[:], wT[128 * k:128 * (k + 1), :])
            w_sb.append(t_)
        u_sb = []
        for k in range(KI):
            t_ = const.tile([128, HIDDEN], F32, tag=f"u{k}")
            nc.sync.dma_start(t_[:], uT[128 * k:128 * (k + 1), :])
            u_sb.append(t_)
        v_sb = []
        for k in range(KH):
            t_ = const.tile([128, HALF], F32, tag=f"v{k}")
            nc.sync.dma_start(t_[:], vT[128 * k:128 * (k + 1), :])
            v_sb.append(t_)
        c_sb = []
        for k in range(KO):
            t_ = const.tile([128, OUT], F32, tag=f"c{k}")
            nc.sync.dma_start(t_[:], cT[128 * k:128 * (k + 1), :])
            c_sb.append(t_)

        ubwb_sb = const.tile([128, KH], F32, tag="ubwb")
        nc.sync.dma_start(ubwb_sb[:], ubwb[:])
        vb_sb = const.tile([128, KO], F32, tag="vb")
        nc.sync.dma_start(vb_sb[:], vb[:])
        cb_sb = const.tile([OUT, 1], F32, tag="cb")
        nc.sync.dma_start(cb_sb[:], cb[:])

        # ---------------- g pre-phase:  g = U x + (u_b + w_b) ----------------
        with ExitStack() as gctx:
            xpool = gctx.enter_context(tc.tile_pool(name="xc", bufs=2))
            gstage_pool = gctx.enter_context(tc.tile_pool(name="gstage", bufs=2))
            gps_pool = gctx.enter_context(
                tc.tile_pool(name="gps", bufs=4, space="PSUM"))
            ncols = GCHUNK * RPC  # 512
            for ci in range(ngch):
                xc = xpool.tile([128, KI, ncols], F32, tag="xc")
                for ic in range(KI):
                    nc.sync.dma_start(
                        xc[:, ic, :],
                        xT[ic, :, ci * ncols:(ci + 1) * ncols])
                gstage = gstage_pool.tile([128, GCHUNK, KH * RPC], F32,
                                          tag="gstage")
                for m in range(KH):
                    ps = gps_pool.tile([128, ncols], F32, tag="gps")
                    for ic in range(KI):
                        nc.tensor.matmul(
                            ps[:],
                            u_sb[ic][:, 128 * m:128 * (m + 1)],
                            xc[:, ic, :],
                            start=(ic == 0), stop=(ic == KI - 1))
                    nc.scalar.activation(
                        gstage[:, :, RPC * m:RPC * (m + 1)],
                        ps[:].rearrange("p (t b) -> p t b", b=RPC),
                        mybir.ActivationFunctionType.Identity,
                        bias=ubwb_sb[:, m:m + 1])
                for ti in range(GCHUNK):
                    nc.sync.dma_start(g_dram[ci * GCHUNK + ti],
                                      gstage[:, ti, :])

        tc.strict_bb_all_engine_barrier()

        # ---------------- recurrence scan ----------------
        hpool = ctx.enter_context(tc.tile_pool(name="h", bufs=2))
        zpool = ctx.enter_context(tc.tile_pool(name="z", bufs=2))
        gpool = ctx.enter_context(tc.tile_pool(name="g", bufs=8))
        spool = ctx.enter_context(tc.tile_pool(name="snapall", bufs=1))
        pspool = ctx.enter_context(
            tc.tile_pool(name="ps", bufs=2, space="PSUM"))

        snapall = spool.tile([128, NCORES, KH * RPC], F32, tag="snapall")

        h_prev = hpool.tile([128, KH * RPC], BF16, tag="h")
        nc.vector.memset(h_prev[:], 0.0)

        for t in range(nsteps):
            gt = gpool.tile([128, KH * RPC], F32, tag="g")
            nc.sync.dma_start(gt[:], g_dram[t])

            ps = pspool.tile([128, KH * RPC], F32, tag="ps")
            for m in range(KH):
                for k in range(KH):
                    nc.tensor.matmul(
                        ps[:, RPC * m:RPC * (m + 1)],
                        w_sb[k][:, 128 * m:128 * (m + 1)],
                        h_prev[:, RPC * k:RPC * (k + 1)],
                        start=(k == 0), stop=(k == KH - 1))

            z = zpool.tile([128, KH * RPC], F32, tag="z")
            nc.vector.tensor_add(z[:], ps[:], gt[:])
            h_cur = hpool.tile([128, KH * RPC], BF16, tag="h")
            nc.scalar.activation(h_cur[:], z[:],
                                 mybir.ActivationFunctionType.Tanh)

            if t % 2 == 1:
                gj = (T - 1 - t) // 2          # global finishing row idx
                c = gj % RPC                   # local column
                e = (T - 1 - t) // 64          # epoch = owning core
                z_cols = z[:].rearrange("p (m b) -> p m b", b=RPC)[:, :, c]
                snap_cols = snapall[:].rearrange(
                    "p e (m b) -> p e m b", b=RPC)[:, e, :, c]
                nc.scalar.activation(snap_cols, z_cols,
                                     mybir.ActivationFunctionType.Tanh)
            h_prev = h_cur

        # ---------------- epilogue: o = sigmoid(V h + vb), y = C o + cb ------
        ops_pool = ctx.enter_context(
            tc.tile_pool(name="ops", bufs=2, space="PSUM"))
        yps_pool = ctx.enter_context(
            tc.tile_pool(name="yps", bufs=2, space="PSUM"))
        opool = ctx.enter_context(tc.tile_pool(name="o", bufs=2))
        ypool = ctx.enter_context(tc.tile_pool(name="ysb", bufs=2))

        for e in range(NCORES):
            o_sb = opool.tile([128, KO, RPC], F32, tag="o")
            for oc in range(KO):
                ps_o = ops_pool.tile([128, RPC], F32, tag="ops")
                for hk in range(KH):
                    nc.tensor.matmul(
                        ps_o[:],
                        v_sb[hk][:, 128 * oc:128 * (oc + 1)],
                        snapall[:, e, RPC * hk:RPC * (hk + 1)],
                        start=(hk == 0), stop=(hk == KH - 1))
                nc.scalar.activation(
                    o_sb[:, oc, :], ps_o[:],
                    mybir.ActivationFunctionType.Sigmoid,
                    bias=vb_sb[:, oc:oc + 1])
            ps_y = yps_pool.tile([OUT, RPC], F32, tag="yps")
            for oc in range(KO):
                nc.tensor.matmul(
                    ps_y[:], c_sb[oc][:], o_sb[:, oc, :],
                    start=(oc == 0), stop=(oc == KO - 1))
            y_sb = ypool.tile([OUT, RPC], F32, tag="ysb")
            nc.scalar.activation(y_sb[:], ps_y[:],
                                 mybir.ActivationFunctionType.Identity,
                                 bias=cb_sb[:])
            nc.sync.dma_start(y_out[e], y_sb[:])

        for e in range(NCORES):
            nc.sync.dma_start(snap_out[e], snapall[:, e, :])

    nc.compile()
    return nc


class Runner:
    """Compile-once / run-many executor for the bass program via axon PJRT.

    Mirrors concourse.bass2jax.run_bass_via_pjrt's multi-core path, but
    caches the jitted callable and keeps inputs device-resident so repeated
    executions measure device time + dispatch only.  Donation is dropped:
    this kernel writes every element of every ExternalOutput.
    """

    def __init__(self, nc, n_cores=NCORES):
        import jax
        from jax.sharding import Mesh, PartitionSpec
        from jax.experimental.shard_map import shard_map
        from concourse import bass2jax

        bass2jax.install_neuronx_cc_hook()
        self.jax = jax
        self.nc = nc
        self.n_cores = n_cores
        in_names, out_names, out_avals = [], [], []
        partition_name = (nc.partition_id_tensor.name
                          if nc.partition_id_tensor else None)
        for alloc in nc.m.functions[0].allocations:
            if not isinstance(alloc, mybir.MemoryLocationSet):
                continue
            name = alloc.memorylocations[0].name
            if alloc.kind == "ExternalInput":
                if name != partition_name:
                    in_names.append(name)
            elif alloc.kind == "ExternalOutput":
                out_names.append(name)
                out_avals.append(jax.core.ShapedArray(
                    tuple(alloc.tensor_shape), mybir.dt.np(alloc.dtype)))
        self.in_names, self.out_names, self.out_avals = (
            in_names, out_names, out_avals)
        n_params = len(in_names)
        all_in_names = list(in_names) + list(out_names)
        if partition_name is not None:
            all_in_names.append(partition_name)

        def _body(*args):
            operands = list(args)
            if partition_name is not None:
                operands.append(bass2jax.partition_id_tensor())
            outs = bass2jax._bass_exec_p.bind(
                *operands,
                out_avals=tuple(out_avals),
                in_names=tuple(all_in_names),
                out_names=tuple(out_names),
                lowering_input_output_aliases=(),
                sim_require_finite=True,
                sim_require_nnan=True,
                nc=nc,
            )
            return tuple(outs)

        self.devices = jax.devices()[:n_cores]
        self.mesh = Mesh(np.asarray(self.devices), ("core",))
        self._pspec = PartitionSpec("core")
        n_all = n_params + len(out_names)
        self.fn = jax.jit(
            shard_map(_body, mesh=self.mesh,
                      in_specs=(PartitionSpec("core"),) * n_all,
                      out_specs=(PartitionSpec("core"),) * len(out_names),
                      check_rep=False),
            keep_unused=True,
        )
        self._dev_args = None

    def set_inputs(self, in_maps):
        from jax.sharding import NamedSharding
        sh = NamedSharding(self.mesh, self._pspec)
        args = []
        for name in self.in_names:
            cat = np.concatenate([np.asarray(m[name]) for m in in_maps], axis=0)
            args.append(self.jax.device_put(cat, sh))
        for av in self.out_avals:
            z = np.zeros((self.n_cores * av.shape[0], *av.shape[1:]), av.dtype)
            args.append(self.jax.device_put(z, sh))
        self.jax.block_until_ready(args)
        self._dev_args = args

    def run(self):
        outs = self.fn(*self._dev_args)
        self.jax.block_until_ready(outs)
        return outs

    def results(self, outs):
        res = []
        for c in range(self.n_cores):
            d = {}
            for i, name in enumerate(self.out_names):
                av = self.out_avals[i]
                d[name] = np.asarray(outs[i]).reshape(
                    self.n_cores, *av.shape)[c]
            res.append(d)
        return res


_CACHED = {}


def _get_runner(nsteps=NSTEPS):
    if nsteps not in _CACHED:
        _CACHED[nsteps] = Runner(_build_program(nsteps))
    return _CACHED[nsteps]


def _host_inputs(x_data, batch_sizes, u_w, u_b, w_w, w_b, v_w, v_b, c_w, c_b,
                 nsteps=NSTEPS):
    bs = np.asarray(batch_sizes, dtype=np.int64)
    offsets = np.concatenate([[0], np.cumsum(bs)[:-1]])
    total = x_data.shape[0]

    common = {
        "wT": np.ascontiguousarray(np.asarray(w_w).T).astype(ml_dtypes.bfloat16),
        "uT": np.ascontiguousarray(np.asarray(u_w).T.astype(np.float32)),
        "vT": np.ascontiguousarray(np.asarray(v_w).T.astype(np.float32)),
        "cT": np.ascontiguousarray(np.asarray(c_w).T.astype(np.float32)),
        "ubwb": np.ascontiguousarray(
            (np.asarray(u_b) + np.asarray(w_b)).astype(np.float32)
            .reshape(KH, 128).T),
        "vb": np.ascontiguousarray(
            np.asarray(v_b).astype(np.float32).reshape(KO, 128).T),
        "cb": np.asarray(c_b).astype(np.float32).reshape(OUT, 1),
    }

    xt_full = np.ascontiguousarray(np.asarray(x_data).T.astype(np.float32))
    in_maps = []
    j_idx = np.arange(RPC)[None, :]
    for k in range(NCORES):
        idx = np.minimum(offsets[:nsteps, None] + 32 * k + j_idx, total - 1)
        xTk = np.ascontiguousarray(
            xt_full[:, idx.reshape(-1)]).reshape(KI, 128, nsteps * RPC)
        m = dict(common)
        m["xT"] = xTk
        in_maps.append(m)
    return in_maps


def kernel(x_data, batch_sizes, sorted_indices, u_w, u_b, w_w, w_b,
           v_w, v_b, c_w, c_b, nsteps=NSTEPS, trace=False):
    runner = _get_runner(nsteps)
    in_maps = _host_inputs(x_data, batch_sizes, u_w, u_b, w_w, w_b,
                           v_w, v_b, c_w, c_b, nsteps=nsteps)
    runner.set_inputs(in_maps)
    results = runner.results(runner.run())

    hidden_sorted = np.zeros((B, HIDDEN), np.float32)
    y_sorted = np.zeros((B, OUT), np.float32)
    for k in range(NCORES):
        snap = np.asarray(results[k]["snap"][k])          # [128, 8*RPC]
        # snap[p, RPC*m + b] = h[b, 128m + p]
        hk = snap.reshape(128, KH, RPC).transpose(2, 1, 0).reshape(RPC, HIDDEN)
        hidden_sorted[32 * k:32 * (k + 1)] = hk
        y_sorted[32 * k:32 * (k + 1)] = np.asarray(results[k]["y"][k]).T

    si = np.asarray(sorted_indices).astype(np.int64)
    hidden = np.zeros((B, HIDDEN), np.float32)
    y = np.zeros((B, OUT), np.float32)
    hidden[si] = hidden_sorted
    y[si] = y_sorted
    return y, hidden
